# revision 6
# baseline (speedup 1.0000x reference)
"""Dice + CrossEntropy loss kernel for Trainium2 (8 NeuronCores, Bass/Tile).

Problem: x (16, 8, 512, 512) f32 logits, y (16, 512, 512) int labels.
    out = dice_loss + ce_loss   (scalar f32)

Sharding: pure data parallel over the batch dim — core j handles batches
[2j, 2j+1]. All cross-batch reductions are tiny and done on the host.

Device layout (per core, per "chunk" of 8192 pixels of one batch image):
  SBUF tile [128, 512] with partition k = c*16 + i (class c in 0..7,
  pixel-block i in 0..15), free dim n in 0..511; pixel = (i, n).
  - ACT: e = exp(x)                                  (bf16)
  - PE (lhsT "w" [128,24]: cols 0..15 block-diag over i, cols 16..23
    class-select over c), rhs=e -> rows 0:16 = s = sumexp per pixel
  - ACT: lns = ln(s); r = exp(-lns) = 1/s            (bf16)
  - DMA: broadcast [y | r] rows 0:16 -> all 128 partitions (3 doubling
    SBUF->SBUF copies; row k must equal row k%16)
  - DVE: p  = e * r_bcast
         me = (y_bcast == c ? 1 : 0) * p             (scalar_tensor_tensor)
  - PE: rhs=me -> rows 0:16 = p@y per pixel, rows 16:24 = tp partials
        rhs=p  -> rows 16:24 = p_sum partials (PSUM-accumulated per batch)
  - ACT: ln(p@y) with accum_out -> CE partial column
  - DVE: acc_tp += tp partials
Host: final tiny reductions + dice/CE formula; label counts via bincount.
"""

import os
import sys

if os.path.isdir("/opt/trn_rl_repo") and "/opt/trn_rl_repo" not in sys.path:
    sys.path.insert(0, "/opt/trn_rl_repo")

import numpy as np
import ml_dtypes

B, C, H, W = 16, 8, 512, 512
HW = H * W
N_CORES = 8
B_LOC = B // N_CORES          # batches per core
SMOOTH = 1e-05
EPS = 1e-08

NCOLS = 512                   # free-dim columns per chunk
IBLK = 16                     # pixel blocks per chunk (partition = c*IBLK+i)
PIX_PER_CHUNK = IBLK * NCOLS  # 8192

_BF16 = ml_dtypes.bfloat16

_cache = {}


def _build_graph(b_loc=B_LOC, hw=HW):
    """Build the per-core Bass graph. Returns finalized nc."""
    import concourse.bacc as bacc
    import concourse.tile as tile
    from concourse import mybir

    chunks_per_b = hw // PIX_PER_CHUNK
    n_chunks = b_loc * chunks_per_b

    nc = bacc.Bacc()
    x_d = nc.dram_tensor("x", [b_loc, C, hw], mybir.dt.float32, kind="ExternalInput")
    y_d = nc.dram_tensor("y", [b_loc, hw], mybir.dt.bfloat16, kind="ExternalInput")
    w_d = nc.dram_tensor("w", [128, 40], mybir.dt.bfloat16, kind="ExternalInput")
    cvec_d = nc.dram_tensor("cvec", [128, 1], mybir.dt.bfloat16, kind="ExternalInput")
    o_tp = nc.dram_tensor("o_tp", [8 * b_loc, NCOLS], mybir.dt.float32,
                          kind="ExternalOutput")
    o_ps = nc.dram_tensor("o_ps", [8 * b_loc, NCOLS], mybir.dt.float32,
                          kind="ExternalOutput")
    o_lp = nc.dram_tensor("o_lp", [IBLK, n_chunks], mybir.dt.float32,
                          kind="ExternalOutput")

    fp32 = mybir.dt.float32
    bf16 = mybir.dt.bfloat16
    Act = mybir.ActivationFunctionType
    Alu = mybir.AluOpType

    with tile.TileContext(nc) as tc:
        with (
            tc.tile_pool(name="singles", bufs=1) as singles,
            tc.tile_pool(name="xin", bufs=3) as xin,
            tc.tile_pool(name="ybuf", bufs=3) as ybuf,
            tc.tile_pool(name="ebuf", bufs=3) as ebuf,
            tc.tile_pool(name="work", bufs=3) as work,
            tc.tile_pool(name="small", bufs=3) as small,
            tc.tile_pool(name="psA", bufs=2, space="PSUM") as psA,
            tc.tile_pool(name="psB", bufs=2, space="PSUM") as psB,
            tc.tile_pool(name="psC", bufs=2, space="PSUM") as psC,
        ):
            w_sb = singles.tile([128, 40], bf16)
            nc.sync.dma_start(out=w_sb, in_=w_d[:, :])
            cvec_sb = singles.tile([128, 1], bf16)
            nc.sync.dma_start(out=cvec_sb, in_=cvec_d[:, :])

            acc_tp = [singles.tile([8, NCOLS], fp32, name=f"acc_tp{b}", tag=f"acc_tp{b}")
                      for b in range(b_loc)]
            for t in acc_tp:
                nc.vector.memset(t, 0.0)
            acc_lp = singles.tile([IBLK, n_chunks], fp32)
            acc_ps = [singles.tile([8, NCOLS], fp32, name=f"acc_ps{b}", tag=f"acc_ps{b}")
                      for b in range(b_loc)]

            ps_acc = None
            for j in range(n_chunks):
                b = j // chunks_per_b
                jj = j % chunks_per_b
                s0 = jj * PIX_PER_CHUNK

                # -- load x chunk: [128 = (c,i), 512] --
                xt = xin.tile([128, NCOLS], fp32)
                for c in range(C):
                    xc = x_d[b, c, s0:s0 + PIX_PER_CHUNK].rearrange(
                        "(i n) -> i n", i=IBLK)
                    nc.sync.dma_start(out=xt[c * IBLK:(c + 1) * IBLK, :], in_=xc)

                # -- load y chunk into cols 0:512 of the [y | r] tile --
                yr = ybuf.tile([128, 2 * NCOLS], bf16)
                yc = y_d[b, s0:s0 + PIX_PER_CHUNK].rearrange(
                    "(i n) -> i n", i=IBLK)
                nc.sync.dma_start(out=yr[0:IBLK, 0:NCOLS], in_=yc)

                # -- e = exp(x) --
                e = ebuf.tile([128, NCOLS], bf16)
                nc.scalar.activation(e, xt, Act.Exp)

                # -- s = sumexp per pixel (PE, rows 0:16) --
                s_ps = psA.tile([40, NCOLS], fp32)
                nc.tensor.matmul(s_ps, w_sb, e, start=True, stop=True)

                # -- r = 1/s via exp(-ln(s)), written into yr cols 512:1024 --
                lns = small.tile([IBLK, NCOLS], fp32)
                nc.scalar.activation(lns, s_ps[0:IBLK, :], Act.Ln)
                nc.scalar.activation(yr[0:IBLK, NCOLS:2 * NCOLS], lns,
                                     Act.Exp, scale=-1.0)

                # -- broadcast rows 0:16 -> 128 (row k = row k%16) --
                nc.sync.dma_start(out=yr[16:32, :], in_=yr[0:16, :])
                nc.sync.dma_start(out=yr[32:64, :], in_=yr[0:32, :])
                nc.sync.dma_start(out=yr[64:128, :], in_=yr[0:64, :])

                # -- p = e * r ; me = (y == c) * p --
                p = work.tile([128, NCOLS], bf16, tag="p")
                nc.vector.tensor_mul(p, e, yr[:, NCOLS:2 * NCOLS])
                me = work.tile([128, NCOLS], bf16, tag="me")
                nc.vector.scalar_tensor_tensor(
                    out=me, in0=yr[:, 0:NCOLS], scalar=cvec_sb, in1=p,
                    op0=Alu.is_equal, op1=Alu.mult)

                # -- pay (rows 0:16) + tp partials (rows 16:24) --
                pm_ps = psB.tile([40, NCOLS], fp32)
                nc.tensor.matmul(pm_ps, w_sb, me, start=True, stop=True)

                # -- p_sum partials accumulate in PSUM across each batch --
                if jj == 0:
                    ps_acc = psC.tile([40, NCOLS], fp32, tag="ps_acc")
                nc.tensor.matmul(ps_acc, w_sb, p,
                                 start=(jj == 0), stop=(jj == chunks_per_b - 1))

                # -- CE partial: sum_n ln(p@y) per i-row -> acc_lp[:, j] --
                lp_scratch = small.tile([IBLK, NCOLS], bf16, tag="lps")
                nc.scalar.activation(lp_scratch, pm_ps[0:IBLK, :], Act.Ln,
                                     accum_out=acc_lp[:, j:j + 1])

                # -- tp accumulate (SBUF, f32) --
                nc.vector.tensor_add(acc_tp[b], pm_ps[32:40, :], acc_tp[b])

                # -- at batch end, move p_sum accumulator out of PSUM --
                if jj == chunks_per_b - 1:
                    nc.vector.tensor_copy(acc_ps[b], ps_acc[32:40, :])

            for b in range(b_loc):
                nc.sync.dma_start(out=o_tp[8 * b:8 * b + 8, :], in_=acc_tp[b])
                nc.sync.dma_start(out=o_ps[8 * b:8 * b + 8, :], in_=acc_ps[b])
            nc.sync.dma_start(out=o_lp[:, :], in_=acc_lp)

    nc.finalize()
    return nc


def _host_constants():
    w = np.zeros((128, 40), dtype=_BF16)
    cvec = np.zeros((128, 1), dtype=_BF16)
    for c in range(C):
        for i in range(IBLK):
            k = c * IBLK + i
            w[k, i] = 1
            w[k, 32 + c] = 1
            cvec[k, 0] = c
    return w, cvec


def kernel(x, y):
    from concourse.bass_utils import run_bass_kernel_spmd

    x = np.asarray(x, dtype=np.float32).reshape(B, C, HW)
    y_int = np.asarray(y).reshape(B, HW)
    y_bf = y_int.astype(np.float32).astype(_BF16)

    if "nc" not in _cache:
        _cache["nc"] = _build_graph()
    nc = _cache["nc"]

    w, cvec = _host_constants()
    in_maps = [
        {
            "x": x[j * B_LOC:(j + 1) * B_LOC],
            "y": y_bf[j * B_LOC:(j + 1) * B_LOC],
            "w": w,
            "cvec": cvec,
        }
        for j in range(N_CORES)
    ]
    res = run_bass_kernel_spmd(nc, in_maps, core_ids=list(range(N_CORES)))

    # host-side reduction
    counts = np.stack(
        [np.bincount(y_int[b].astype(np.int64), minlength=C) for b in range(B)]
    ).astype(np.float64)                       # (B, C)

    tp = np.zeros((B, C), dtype=np.float64)
    ps = np.zeros((B, C), dtype=np.float64)
    lp_total = 0.0
    for j in range(N_CORES):
        r = res.results[j]
        otp = np.asarray(r["o_tp"], dtype=np.float64)   # (16, 512)
        ops_ = np.asarray(r["o_ps"], dtype=np.float64)
        olp = np.asarray(r["o_lp"], dtype=np.float64)
        for bl in range(B_LOC):
            bg = j * B_LOC + bl
            tp[bg] = otp[8 * bl:8 * bl + 8].sum(axis=1)
            ps[bg] = ops_[8 * bl:8 * bl + 8].sum(axis=1)
        lp_total += olp.sum()

    dc = (2.0 * tp + SMOOTH) / (ps + counts + SMOOTH + EPS)
    dc_loss = 1.0 - dc[:, 1:].mean()
    ce_loss = -lp_total / (B * HW)
    return np.float32(dc_loss + ce_loss)


# revision 12
# speedup vs baseline: 3.2679x; 3.2679x over previous
"""Dice + CrossEntropy loss kernel for Trainium2 (8 NeuronCores, Bass/Tile).

Problem: x (16, 8, 512, 512) f32 logits, y (16, 512, 512) int labels.
    out = dice_loss + ce_loss   (scalar f32)

Sharding: pure data parallel over the batch dim - core j handles batches
[2j, 2j+1]. All cross-core reductions are tiny and done on the host.

Device pipeline (per core). Work unit is a "supergroup" of 8 chunks; a
chunk is 8192 pixels of one batch image on SBUF layout [128 = (c*16+i),
512 = n], class c in 0..7, pixel-block i in 0..15, pixel = (i, n):

  host: x pre-transposed so a supergroup is one contiguous [128, 4096]
        2 MiB DMA; y to bf16, pre-transposed to [16, 4096].
  ACT : e8 = exp(x8)                                   [128,4096] bf16
  PE  : m1(q), q=0..7, zero-padded block-diag lhsT, accumulate all 8
        chunks' sumexp into one PSUM tile s8[128,512] (row 8i+q).
  ACT : lns8 = ln(s8); r8 = exp(-lns8) = 1/s           [128,512]
  DMA : r8 -> DRAM scratch -> permuted into yrg r-half rows 0:16
        (DRAM APs are unconstrained); then 3 doubling SBUF copies
        broadcast [y | r] rows 0:16 -> all 128 partitions.
  DVE : mask8 = (y == c); p8 = e8 * r; me8 = p8 * mask8  [128,4096]
  PE  : m3(t) per 4-chunk pm-group -> pm4[72,512]: rows 4i+t = p@y
        per pixel, rows 64:72 = tp partials; m4(q) -> ps_acc[8,512]
        p_sum partials accumulated across each batch.
  ACT : ln(pm4 rows 0:64) with accum_out -> CE partial column
  DVE : acc_tp += tp partials; batch end: copy ps_acc out of PSUM
Host: tiny final reductions + dice/CE formula; label counts via bincount.
"""

import os
import sys

if os.path.isdir("/opt/trn_rl_repo") and "/opt/trn_rl_repo" not in sys.path:
    sys.path.insert(0, "/opt/trn_rl_repo")

import numpy as np
import ml_dtypes

B, C, H, W = 16, 8, 512, 512
HW = H * W
N_CORES = 8
B_LOC = B // N_CORES          # batches per core
SMOOTH = 1e-05
EPS = 1e-08

NCOLS = 512                   # free-dim columns per chunk
IBLK = 16                     # pixel blocks per chunk
PIX_PER_CHUNK = IBLK * NCOLS  # 8192
QS = 8                        # chunks per supergroup
SGCOLS = QS * NCOLS           # 4096
PIX_PER_SG = QS * PIX_PER_CHUNK  # 65536
QM = 4                        # chunks per pm-group (pay/tp matmul group)

_BF16 = ml_dtypes.bfloat16

_cache = {}

# wpack column layout
WS0 = 0                # 8 blocks of [128,128] at cols 128*q
WM0 = 8 * 128          # 4 blocks of [128,72] at cols WM0 + 72*t
WP0 = WM0 + 4 * 72     # [128,8]
WCOLS = WP0 + 8        # 1320


def _patch_act_tables():
    """Force every activation onto the one table set that contains both
    Exp and Ln, so the kernel needs a single ACT_TABLE_LOAD instead of
    thrashing tables between Exp and Ln every chunk. Index order of the
    sets is preserved (ids must match act_info.json); unwanted sets are
    just emptied so the chooser can't pick them."""
    from concourse import hw_specs
    import concourse.bacc as bacc_mod

    if getattr(hw_specs, "_act_tables_patched", False):
        return
    orig = hw_specs.get_activation_tables

    def patched(arch):
        tables = orig(arch)
        keep = "natural_log_exp_and_others"
        if keep in tables:
            tables = {
                name: (funcs if name == keep else set())
                for name, funcs in tables.items()
            }
        return tables

    hw_specs.get_activation_tables = patched
    bacc_mod.get_activation_tables = patched
    hw_specs._act_tables_patched = True


def _build_graph(b_loc=B_LOC, hw=HW):
    """Build the per-core Bass graph. Returns finalized nc."""
    _patch_act_tables()
    import concourse.bacc as bacc
    import concourse.tile as tile
    from concourse import mybir

    sg_per_b = hw // PIX_PER_SG
    n_sg = b_loc * sg_per_b
    n_mg = n_sg * 2               # pm-groups

    nc = bacc.Bacc()
    x_d = nc.dram_tensor("x", [b_loc, sg_per_b, 128, SGCOLS],
                         mybir.dt.float32, kind="ExternalInput")
    y_d = nc.dram_tensor("y", [b_loc, sg_per_b, IBLK, SGCOLS],
                         mybir.dt.bfloat16, kind="ExternalInput")
    w_d = nc.dram_tensor("w", [128, WCOLS], mybir.dt.bfloat16,
                         kind="ExternalInput")
    cvec_d = nc.dram_tensor("cvec", [128, 1], mybir.dt.float32,
                            kind="ExternalInput")
    o_tp = nc.dram_tensor("o_tp", [8 * b_loc, NCOLS], mybir.dt.float32,
                          kind="ExternalOutput")
    o_ps = nc.dram_tensor("o_ps", [8 * b_loc, NCOLS], mybir.dt.float32,
                          kind="ExternalOutput")
    o_lp = nc.dram_tensor("o_lp", [64, n_mg], mybir.dt.float32,
                          kind="ExternalOutput")

    fp32 = mybir.dt.float32
    bf16 = mybir.dt.bfloat16
    Act = mybir.ActivationFunctionType
    Alu = mybir.AluOpType

    with tile.TileContext(nc) as tc:
        with (
            tc.tile_pool(name="singles", bufs=1) as singles,
            tc.tile_pool(name="xin", bufs=2) as xin,
            tc.tile_pool(name="ybuf", bufs=2) as ybuf,
            tc.tile_pool(name="ebuf", bufs=2) as ebuf,
            tc.tile_pool(name="work", bufs=2) as work,
            tc.tile_pool(name="small", bufs=3) as small,
            tc.tile_pool(name="rdram", bufs=3, space="DRAM") as rdram,
            tc.tile_pool(name="psA", bufs=2, space="PSUM") as psA,
            tc.tile_pool(name="psB", bufs=2, space="PSUM") as psB,
            tc.tile_pool(name="psC", bufs=2, space="PSUM") as psC,
        ):
            w_sb = singles.tile([128, WCOLS], bf16)
            nc.sync.dma_start(out=w_sb, in_=w_d[:, :])
            cvec_sb = singles.tile([128, 1], fp32)
            nc.sync.dma_start(out=cvec_sb, in_=cvec_d[:, :])

            acc_tp = [singles.tile([8, NCOLS], fp32, name=f"acc_tp{b}")
                      for b in range(b_loc)]
            for t in acc_tp:
                nc.vector.memset(t, 0.0)
            acc_lp = singles.tile([64, n_mg], fp32)
            acc_ps = [singles.tile([8, NCOLS], fp32, name=f"acc_ps{b}")
                      for b in range(b_loc)]

            ps_acc = None
            for sgi in range(n_sg):
                b = sgi // sg_per_b
                sg = sgi % sg_per_b

                # -- load x supergroup (one contiguous 2 MiB DMA) --
                xt8 = xin.tile([128, SGCOLS], fp32)
                nc.sync.dma_start(out=xt8, in_=x_d[b, sg])

                # -- load y supergroup into cols 0:4096 of [y | r] --
                yrg = ybuf.tile([128, 2 * SGCOLS], bf16)
                nc.sync.dma_start(out=yrg[0:IBLK, 0:SGCOLS], in_=y_d[b, sg])

                # -- e8 = exp(x8) --
                e8 = ebuf.tile([128, SGCOLS], bf16)
                nc.scalar.activation(e8, xt8, Act.Exp)

                # -- s8[8i+q, n] = sumexp, 8 accumulated matmuls --
                s8 = psA.tile([128, NCOLS], fp32)
                for q in range(QS):
                    nc.tensor.matmul(
                        s8, w_sb[:, WS0 + 128 * q:WS0 + 128 * (q + 1)],
                        e8[:, NCOLS * q:NCOLS * (q + 1)],
                        start=(q == 0), stop=(q == QS - 1))

                # -- r8 = 1/s8 via exp(-ln) --
                lns8 = small.tile([128, NCOLS], fp32)
                nc.scalar.activation(lns8, s8, Act.Ln)
                r8 = small.tile([128, NCOLS], bf16, tag="r8")
                nc.scalar.activation(r8, lns8, Act.Exp, scale=-1.0)

                # -- r8 -> DRAM -> permuted into yrg rows 0:16 r-half --
                r_dram = rdram.tile([128, NCOLS], bf16, name="r_dram")
                nc.gpsimd.dma_start(out=r_dram, in_=r8)
                nc.sync.dma_start(
                    out=yrg[0:IBLK, SGCOLS:2 * SGCOLS].rearrange(
                        "i (q n) -> i q n", q=QS),
                    in_=r_dram.rearrange("(i q) n -> i q n", q=QS))

                # -- doubling copies broadcast rows 0:16 -> all 128 --
                nc.gpsimd.dma_start(out=yrg[16:32, :], in_=yrg[0:16, :])
                nc.gpsimd.dma_start(out=yrg[32:64, :], in_=yrg[0:32, :])
                nc.sync.dma_start(out=yrg[64:128, :], in_=yrg[0:64, :])

                # -- mask, p, me over the supergroup [128, 4096] --
                mask8 = work.tile([128, SGCOLS], bf16, tag="mask8")
                nc.vector.tensor_scalar(
                    out=mask8, in0=yrg[:, 0:SGCOLS], scalar1=cvec_sb,
                    scalar2=None, op0=Alu.is_equal)
                p8 = work.tile([128, SGCOLS], bf16, tag="p8")
                nc.vector.tensor_mul(p8, e8, yrg[:, SGCOLS:2 * SGCOLS])
                me8 = work.tile([128, SGCOLS], bf16, tag="me8")
                nc.vector.tensor_mul(me8, p8, mask8)

                # -- pay + tp partials per pm-group of 4 chunks --
                for mg in range(2):
                    pm4 = psB.tile([72, NCOLS], fp32, tag="pm4")
                    for t in range(QM):
                        cq = mg * QM + t
                        nc.tensor.matmul(
                            pm4, w_sb[:, WM0 + 72 * t:WM0 + 72 * (t + 1)],
                            me8[:, NCOLS * cq:NCOLS * (cq + 1)],
                            start=(t == 0), stop=(t == QM - 1))
                    lp_scratch = small.tile([64, NCOLS], bf16, tag="lps")
                    nc.scalar.activation(
                        lp_scratch, pm4[0:64, :], Act.Ln,
                        accum_out=acc_lp[:, 2 * sgi + mg:2 * sgi + mg + 1])
                    nc.vector.tensor_add(acc_tp[b], pm4[64:72, :], acc_tp[b])

                # -- p_sum partials accumulate in PSUM across each batch --
                if sg == 0:
                    ps_acc = psC.tile([8, NCOLS], fp32, tag="ps_acc")
                for q in range(QS):
                    nc.tensor.matmul(
                        ps_acc, w_sb[:, WP0:WP0 + 8],
                        p8[:, NCOLS * q:NCOLS * (q + 1)],
                        start=(sg == 0 and q == 0),
                        stop=(sg == sg_per_b - 1 and q == QS - 1))

                # -- at batch end, move p_sum accumulator out of PSUM --
                if sg == sg_per_b - 1:
                    nc.vector.tensor_copy(acc_ps[b], ps_acc)

            for b in range(b_loc):
                nc.sync.dma_start(out=o_tp[8 * b:8 * b + 8, :], in_=acc_tp[b])
                nc.sync.dma_start(out=o_ps[8 * b:8 * b + 8, :], in_=acc_ps[b])
            nc.sync.dma_start(out=o_lp[:, :], in_=acc_lp)

    nc.finalize()
    return nc


def _host_constants():
    w = np.zeros((128, WCOLS), dtype=_BF16)
    cvec = np.zeros((128, 1), dtype=np.float32)
    for c in range(C):
        for i in range(IBLK):
            k = c * IBLK + i
            cvec[k, 0] = c
            for q in range(QS):
                w[k, WS0 + 128 * q + 8 * i + q] = 1    # ws_q block-diag
            for t in range(QM):
                w[k, WM0 + 72 * t + 4 * i + t] = 1     # wm_t block-diag
                w[k, WM0 + 72 * t + 64 + c] = 1        # wm_t class-sel
            w[k, WP0 + c] = 1                          # wp class-sel
    return w, cvec


def _prep_x(x, hw):
    sg_per_b = hw // PIX_PER_SG
    nb = x.shape[0]
    xr = x.reshape(nb, C, sg_per_b, QS, IBLK, NCOLS)
    return np.ascontiguousarray(
        xr.transpose(0, 2, 1, 4, 3, 5)).reshape(nb, sg_per_b, 128, SGCOLS)


def _prep_y(y_bf, hw):
    sg_per_b = hw // PIX_PER_SG
    nb = y_bf.shape[0]
    yr = y_bf.reshape(nb, sg_per_b, QS, IBLK, NCOLS)
    return np.ascontiguousarray(
        yr.transpose(0, 1, 3, 2, 4)).reshape(nb, sg_per_b, IBLK, SGCOLS)


def kernel(x, y):
    from concourse.bass_utils import run_bass_kernel_spmd

    x = np.asarray(x, dtype=np.float32).reshape(B, C, HW)
    y_int = np.asarray(y).reshape(B, HW)
    y_bf = y_int.astype(np.float32).astype(_BF16)

    if "nc" not in _cache:
        _cache["nc"] = _build_graph()
    nc = _cache["nc"]

    w, cvec = _host_constants()
    x_dev = _prep_x(x, HW)
    y_dev = _prep_y(y_bf, HW)
    in_maps = [
        {
            "x": x_dev[j * B_LOC:(j + 1) * B_LOC],
            "y": y_dev[j * B_LOC:(j + 1) * B_LOC],
            "w": w,
            "cvec": cvec,
        }
        for j in range(N_CORES)
    ]
    res = run_bass_kernel_spmd(nc, in_maps, core_ids=list(range(N_CORES)))

    # host-side reduction
    counts = np.stack(
        [np.bincount(y_int[b].astype(np.int64), minlength=C) for b in range(B)]
    ).astype(np.float64)                       # (B, C)

    tp = np.zeros((B, C), dtype=np.float64)
    ps = np.zeros((B, C), dtype=np.float64)
    lp_total = 0.0
    for j in range(N_CORES):
        r = res.results[j]
        otp = np.asarray(r["o_tp"], dtype=np.float64)   # (16, 512)
        ops_ = np.asarray(r["o_ps"], dtype=np.float64)
        olp = np.asarray(r["o_lp"], dtype=np.float64)
        for bl in range(B_LOC):
            bg = j * B_LOC + bl
            tp[bg] = otp[8 * bl:8 * bl + 8].sum(axis=1)
            ps[bg] = ops_[8 * bl:8 * bl + 8].sum(axis=1)
        lp_total += olp.sum()

    dc = (2.0 * tp + SMOOTH) / (ps + counts + SMOOTH + EPS)
    dc_loss = 1.0 - dc[:, 1:].mean()
    ce_loss = -lp_total / (B * HW)
    return np.float32(dc_loss + ce_loss)


# revision 14
# speedup vs baseline: 3.3254x; 1.0176x over previous
"""Dice + CrossEntropy loss kernel for Trainium2 (8 NeuronCores, Bass/Tile).

Problem: x (16, 8, 512, 512) f32 logits, y (16, 512, 512) int labels.
    out = dice_loss + ce_loss   (scalar f32)

Sharding: pure data parallel over the batch dim - core j handles batches
[2j, 2j+1]. All cross-core reductions are tiny and done on the host.

Device pipeline (per core). Work unit is a "supergroup" of 8 chunks; a
chunk is 8192 pixels of one batch image on SBUF layout [128 = (c*16+i),
512 = n], class c in 0..7, pixel-block i in 0..15, pixel = (i, n):

  host: x pre-transposed so a supergroup is one contiguous [128, 4096]
        2 MiB DMA; y to bf16, pre-transposed to [16, 4096].
  ACT : e8 = exp(x8)                                   [128,4096] bf16
  PE  : m1(q), q=0..7, zero-padded block-diag lhsT, accumulate all 8
        chunks' sumexp into one PSUM tile s8[128,512] (row 8i+q).
  ACT : lns8 = ln(s8); r8 = exp(-lns8) = 1/s           [128,512]
  DMA : r8 -> DRAM scratch -> permuted into yrg r-half rows 0:16
        (DRAM APs are unconstrained); then 3 doubling SBUF copies
        broadcast [y | r] rows 0:16 -> all 128 partitions.
  DVE : mask8 = (y == c); p8 = e8 * r; me8 = p8 * mask8  [128,4096]
  PE  : m3(t) per 4-chunk pm-group -> pm4[72,512]: rows 4i+t = p@y
        per pixel, rows 64:72 = tp partials; m4(q) -> ps_acc[8,512]
        p_sum partials accumulated across each batch.
  ACT : ln(pm4 rows 0:64) with accum_out -> CE partial column
  DVE : acc_tp += tp partials; batch end: copy ps_acc out of PSUM
Host: tiny final reductions + dice/CE formula; label counts via bincount.
"""

import os
import sys

if os.path.isdir("/opt/trn_rl_repo") and "/opt/trn_rl_repo" not in sys.path:
    sys.path.insert(0, "/opt/trn_rl_repo")

import numpy as np
import ml_dtypes

B, C, H, W = 16, 8, 512, 512
HW = H * W
N_CORES = 8
B_LOC = B // N_CORES          # batches per core
SMOOTH = 1e-05
EPS = 1e-08

NCOLS = 512                   # free-dim columns per chunk
IBLK = 16                     # pixel blocks per chunk
PIX_PER_CHUNK = IBLK * NCOLS  # 8192
QS = 8                        # chunks per supergroup
SGCOLS = QS * NCOLS           # 4096
PIX_PER_SG = QS * PIX_PER_CHUNK  # 65536
QM = 4                        # chunks per pm-group (pay/tp matmul group)

_BF16 = ml_dtypes.bfloat16

_cache = {}

# wpack column layout
WS0 = 0                # 8 blocks of [128,128] at cols 128*q
WM0 = 8 * 128          # 4 blocks of [128,72] at cols WM0 + 72*t
WP0 = WM0 + 4 * 72     # [128,8]
WCOLS = WP0 + 8        # 1320


def _patch_act_tables():
    """Force every activation onto the one table set that contains both
    Exp and Ln, so the kernel needs a single ACT_TABLE_LOAD instead of
    thrashing tables between Exp and Ln every chunk. Index order of the
    sets is preserved (ids must match act_info.json); unwanted sets are
    just emptied so the chooser can't pick them."""
    from concourse import hw_specs
    import concourse.bacc as bacc_mod

    if getattr(hw_specs, "_act_tables_patched", False):
        return
    orig = hw_specs.get_activation_tables

    def patched(arch):
        tables = orig(arch)
        keep = "natural_log_exp_and_others"
        if keep in tables:
            tables = {
                name: (funcs if name == keep else set())
                for name, funcs in tables.items()
            }
        return tables

    hw_specs.get_activation_tables = patched
    bacc_mod.get_activation_tables = patched
    hw_specs._act_tables_patched = True


def _build_graph(b_loc=B_LOC, hw=HW):
    """Build the per-core Bass graph. Returns finalized nc."""
    _patch_act_tables()
    import concourse.bacc as bacc
    import concourse.tile as tile
    from concourse import mybir

    sg_per_b = hw // PIX_PER_SG
    n_sg = b_loc * sg_per_b
    n_mg = n_sg * 2               # pm-groups

    nc = bacc.Bacc()
    x_d = nc.dram_tensor("x", [b_loc, sg_per_b, 128, SGCOLS],
                         mybir.dt.float32, kind="ExternalInput")
    y_d = nc.dram_tensor("y", [b_loc, sg_per_b, IBLK, SGCOLS],
                         mybir.dt.bfloat16, kind="ExternalInput")
    w_d = nc.dram_tensor("w", [128, WCOLS], mybir.dt.bfloat16,
                         kind="ExternalInput")
    cvec_d = nc.dram_tensor("cvec", [128, 1], mybir.dt.float32,
                            kind="ExternalInput")
    o_tp = nc.dram_tensor("o_tp", [8 * b_loc, NCOLS], mybir.dt.float32,
                          kind="ExternalOutput")
    o_ps = nc.dram_tensor("o_ps", [8 * b_loc, NCOLS], mybir.dt.float32,
                          kind="ExternalOutput")
    o_lp = nc.dram_tensor("o_lp", [64, n_mg], mybir.dt.float32,
                          kind="ExternalOutput")

    fp32 = mybir.dt.float32
    bf16 = mybir.dt.bfloat16
    Act = mybir.ActivationFunctionType
    Alu = mybir.AluOpType

    with tile.TileContext(nc) as tc:
        with (
            tc.tile_pool(name="singles", bufs=1) as singles,
            tc.tile_pool(name="xin", bufs=2) as xin,
            tc.tile_pool(name="ybuf", bufs=3) as ybuf,
            tc.tile_pool(name="ebuf", bufs=2) as ebuf,
            tc.tile_pool(name="work", bufs=2) as work,
            tc.tile_pool(name="small", bufs=4) as small,
            tc.tile_pool(name="rdram", bufs=3, space="DRAM") as rdram,
            tc.tile_pool(name="psA", bufs=2, space="PSUM") as psA,
            tc.tile_pool(name="psB", bufs=2, space="PSUM") as psB,
            tc.tile_pool(name="psC", bufs=2, space="PSUM") as psC,
        ):
            w_sb = singles.tile([128, WCOLS], bf16)
            nc.sync.dma_start(out=w_sb, in_=w_d[:, :])
            cvec_sb = singles.tile([128, 1], fp32)
            nc.sync.dma_start(out=cvec_sb, in_=cvec_d[:, :])

            acc_tp = [singles.tile([8, NCOLS], fp32, name=f"acc_tp{b}")
                      for b in range(b_loc)]
            for t in acc_tp:
                nc.vector.memset(t, 0.0)
            acc_lp = singles.tile([64, n_mg], fp32)
            acc_ps = [singles.tile([8, NCOLS], fp32, name=f"acc_ps{b}")
                      for b in range(b_loc)]

            ps_acc = None
            for sgi in range(n_sg):
                b = sgi // sg_per_b
                sg = sgi % sg_per_b

                # -- load x supergroup (one contiguous 2 MiB DMA) --
                xt8 = xin.tile([128, SGCOLS], fp32)
                nc.sync.dma_start(out=xt8, in_=x_d[b, sg])

                # -- load y supergroup into cols 0:4096 of [y | r] --
                yrg = ybuf.tile([128, 2 * SGCOLS], bf16)
                nc.sync.dma_start(out=yrg[0:IBLK, 0:SGCOLS], in_=y_d[b, sg])

                # -- e8 = exp(x8) --
                e8 = ebuf.tile([128, SGCOLS], bf16)
                nc.scalar.activation(e8, xt8, Act.Exp)

                # -- s8[8i+q, n] = sumexp, 8 accumulated matmuls --
                s8 = psA.tile([128, NCOLS], fp32)
                for q in range(QS):
                    nc.tensor.matmul(
                        s8, w_sb[:, WS0 + 128 * q:WS0 + 128 * (q + 1)],
                        e8[:, NCOLS * q:NCOLS * (q + 1)],
                        start=(q == 0), stop=(q == QS - 1))

                # -- r8 = 1/s8 via exp(-ln) --
                lns8 = small.tile([128, NCOLS], fp32)
                nc.scalar.activation(lns8, s8, Act.Ln)
                r8 = small.tile([128, NCOLS], bf16, tag="r8")
                nc.scalar.activation(r8, lns8, Act.Exp, scale=-1.0)

                # -- r8 -> DRAM -> permuted into yrg rows 0:16 r-half --
                r_dram = rdram.tile([128, NCOLS], bf16, name="r_dram")
                nc.gpsimd.dma_start(out=r_dram, in_=r8)
                # r_dram rows are (8i+q), so its flat [16, 4096] view IS
                # the [i, (q n)] layout the r-half wants -- plain 2D copy.
                nc.sync.dma_start(
                    out=yrg[0:IBLK, SGCOLS:2 * SGCOLS],
                    in_=r_dram.rearrange("(i q) n -> i (q n)", q=QS))

                # -- doubling copies broadcast rows 0:16 -> all 128 --
                nc.gpsimd.dma_start(out=yrg[16:32, :], in_=yrg[0:16, :])
                nc.gpsimd.dma_start(out=yrg[32:64, :], in_=yrg[0:32, :])
                nc.gpsimd.dma_start(out=yrg[64:128, :], in_=yrg[0:64, :])

                # -- mask, p, me over the supergroup [128, 4096] --
                mask8 = work.tile([128, SGCOLS], bf16, tag="mask8")
                nc.vector.tensor_scalar(
                    out=mask8, in0=yrg[:, 0:SGCOLS], scalar1=cvec_sb,
                    scalar2=None, op0=Alu.is_equal)
                p8 = work.tile([128, SGCOLS], bf16, tag="p8")
                nc.vector.tensor_mul(p8, e8, yrg[:, SGCOLS:2 * SGCOLS])
                me8 = work.tile([128, SGCOLS], bf16, tag="me8")
                nc.vector.tensor_mul(me8, p8, mask8)

                # -- pay + tp partials per pm-group of 4 chunks --
                for mg in range(2):
                    pm4 = psB.tile([72, NCOLS], fp32, tag="pm4")
                    for t in range(QM):
                        cq = mg * QM + t
                        nc.tensor.matmul(
                            pm4, w_sb[:, WM0 + 72 * t:WM0 + 72 * (t + 1)],
                            me8[:, NCOLS * cq:NCOLS * (cq + 1)],
                            start=(t == 0), stop=(t == QM - 1))
                    lp_scratch = small.tile([64, NCOLS], bf16, tag="lps")
                    nc.scalar.activation(
                        lp_scratch, pm4[0:64, :], Act.Ln,
                        accum_out=acc_lp[:, 2 * sgi + mg:2 * sgi + mg + 1])
                    nc.vector.tensor_add(acc_tp[b], pm4[64:72, :], acc_tp[b])

                # -- p_sum partials accumulate in PSUM across each batch --
                if sg == 0:
                    ps_acc = psC.tile([8, NCOLS], fp32, tag="ps_acc")
                for q in range(QS):
                    nc.tensor.matmul(
                        ps_acc, w_sb[:, WP0:WP0 + 8],
                        p8[:, NCOLS * q:NCOLS * (q + 1)],
                        start=(sg == 0 and q == 0),
                        stop=(sg == sg_per_b - 1 and q == QS - 1))

                # -- at batch end, move p_sum accumulator out of PSUM --
                if sg == sg_per_b - 1:
                    nc.vector.tensor_copy(acc_ps[b], ps_acc)

            for b in range(b_loc):
                nc.sync.dma_start(out=o_tp[8 * b:8 * b + 8, :], in_=acc_tp[b])
                nc.sync.dma_start(out=o_ps[8 * b:8 * b + 8, :], in_=acc_ps[b])
            nc.sync.dma_start(out=o_lp[:, :], in_=acc_lp)

    nc.finalize()
    return nc


def _host_constants():
    w = np.zeros((128, WCOLS), dtype=_BF16)
    cvec = np.zeros((128, 1), dtype=np.float32)
    for c in range(C):
        for i in range(IBLK):
            k = c * IBLK + i
            cvec[k, 0] = c
            for q in range(QS):
                w[k, WS0 + 128 * q + 8 * i + q] = 1    # ws_q block-diag
            for t in range(QM):
                w[k, WM0 + 72 * t + 4 * i + t] = 1     # wm_t block-diag
                w[k, WM0 + 72 * t + 64 + c] = 1        # wm_t class-sel
            w[k, WP0 + c] = 1                          # wp class-sel
    return w, cvec


def _prep_x(x, hw):
    sg_per_b = hw // PIX_PER_SG
    nb = x.shape[0]
    xr = x.reshape(nb, C, sg_per_b, QS, IBLK, NCOLS)
    return np.ascontiguousarray(
        xr.transpose(0, 2, 1, 4, 3, 5)).reshape(nb, sg_per_b, 128, SGCOLS)


def _prep_y(y_bf, hw):
    sg_per_b = hw // PIX_PER_SG
    nb = y_bf.shape[0]
    yr = y_bf.reshape(nb, sg_per_b, QS, IBLK, NCOLS)
    return np.ascontiguousarray(
        yr.transpose(0, 1, 3, 2, 4)).reshape(nb, sg_per_b, IBLK, SGCOLS)


def kernel(x, y):
    from concourse.bass_utils import run_bass_kernel_spmd

    x = np.asarray(x, dtype=np.float32).reshape(B, C, HW)
    y_int = np.asarray(y).reshape(B, HW)
    y_bf = y_int.astype(np.float32).astype(_BF16)

    if "nc" not in _cache:
        _cache["nc"] = _build_graph()
    nc = _cache["nc"]

    w, cvec = _host_constants()
    x_dev = _prep_x(x, HW)
    y_dev = _prep_y(y_bf, HW)
    in_maps = [
        {
            "x": x_dev[j * B_LOC:(j + 1) * B_LOC],
            "y": y_dev[j * B_LOC:(j + 1) * B_LOC],
            "w": w,
            "cvec": cvec,
        }
        for j in range(N_CORES)
    ]
    res = run_bass_kernel_spmd(nc, in_maps, core_ids=list(range(N_CORES)))

    # host-side reduction
    counts = np.stack(
        [np.bincount(y_int[b].astype(np.int64), minlength=C) for b in range(B)]
    ).astype(np.float64)                       # (B, C)

    tp = np.zeros((B, C), dtype=np.float64)
    ps = np.zeros((B, C), dtype=np.float64)
    lp_total = 0.0
    for j in range(N_CORES):
        r = res.results[j]
        otp = np.asarray(r["o_tp"], dtype=np.float64)   # (16, 512)
        ops_ = np.asarray(r["o_ps"], dtype=np.float64)
        olp = np.asarray(r["o_lp"], dtype=np.float64)
        for bl in range(B_LOC):
            bg = j * B_LOC + bl
            tp[bg] = otp[8 * bl:8 * bl + 8].sum(axis=1)
            ps[bg] = ops_[8 * bl:8 * bl + 8].sum(axis=1)
        lp_total += olp.sum()

    dc = (2.0 * tp + SMOOTH) / (ps + counts + SMOOTH + EPS)
    dc_loss = 1.0 - dc[:, 1:].mean()
    ce_loss = -lp_total / (B * HW)
    return np.float32(dc_loss + ce_loss)


# revision 18
# speedup vs baseline: 4.0080x; 1.2053x over previous
"""Dice + CrossEntropy loss kernel for Trainium2 (8 NeuronCores, Bass/Tile).

Problem: x (16, 8, 512, 512) f32 logits, y (16, 512, 512) int labels.
    out = dice_loss + ce_loss   (scalar f32)

Sharding: pure data parallel over the batch dim - core j handles batches
[2j, 2j+1]. All cross-core reductions are tiny and done on the host.

Device pipeline (per core). Work unit is a "supergroup" of 8 chunks; a
chunk is 8192 pixels of one batch image on SBUF layout [128 = (c*16+i),
512 = n], class c in 0..7, pixel-block i in 0..15, pixel = (i, n):

  host: x pre-transposed so a supergroup is one contiguous [128, 4096]
        2 MiB DMA; y to bf16, pre-transposed to [16, 4096].
  ACT : e8 = exp(x8)                                   [128,4096] bf16
  PE  : m1(q), q=0..7, zero-padded block-diag lhsT, accumulate all 8
        chunks' sumexp into one PSUM tile s8[128,512] (row 8i+q).
  ACT : lns8 = ln(s8); r8 = exp(-lns8) = 1/s           [128,512]
  DMA : r8 -> DRAM scratch -> permuted into yrg r-half rows 0:16
        (DRAM APs are unconstrained); then 3 doubling SBUF copies
        broadcast [y | r] rows 0:16 -> all 128 partitions.
  DVE : mask8 = (y == c); p8 = e8 * r; me8 = p8 * mask8  [128,4096]
  PE  : m3(t) per 4-chunk pm-group -> pm4[72,512]: rows 4i+t = p@y
        per pixel, rows 64:72 = tp partials; m4(q) -> ps_acc[8,512]
        p_sum partials accumulated across each batch.
  ACT : ln(pm4 rows 0:64) with accum_out -> CE partial column
  DVE : acc_tp += tp partials; batch end: copy ps_acc out of PSUM
Host: tiny final reductions + dice/CE formula; label counts via bincount.
"""

import os
import sys

if os.path.isdir("/opt/trn_rl_repo") and "/opt/trn_rl_repo" not in sys.path:
    sys.path.insert(0, "/opt/trn_rl_repo")

import numpy as np
import ml_dtypes

B, C, H, W = 16, 8, 512, 512
HW = H * W
N_CORES = 8
B_LOC = B // N_CORES          # batches per core
SMOOTH = 1e-05
EPS = 1e-08

NCOLS = 512                   # free-dim columns per chunk
IBLK = 16                     # pixel blocks per chunk
PIX_PER_CHUNK = IBLK * NCOLS  # 8192
QS = 8                        # chunks per supergroup
SGCOLS = QS * NCOLS           # 4096
PIX_PER_SG = QS * PIX_PER_CHUNK  # 65536
QM = 4                        # chunks per pm-group (pay/tp matmul group)

_BF16 = ml_dtypes.bfloat16

_cache = {}

# wpack column layout
WS0 = 0                # 8 blocks of [128,128] at cols 128*q
WM0 = 8 * 128          # 4 blocks of [128,72] at cols WM0 + 72*t
WP0 = WM0 + 4 * 72     # [128,8]
WCOLS = WP0 + 8        # 1320


def _patch_act_tables():
    """Force every activation onto the one table set that contains both
    Exp and Ln, so the kernel needs a single ACT_TABLE_LOAD instead of
    thrashing tables between Exp and Ln every chunk. Index order of the
    sets is preserved (ids must match act_info.json); unwanted sets are
    just emptied so the chooser can't pick them."""
    from concourse import hw_specs
    import concourse.bacc as bacc_mod

    if getattr(hw_specs, "_act_tables_patched", False):
        return
    orig = hw_specs.get_activation_tables

    def patched(arch):
        tables = orig(arch)
        keep = "natural_log_exp_and_others"
        if keep in tables:
            tables = {
                name: (funcs if name == keep else set())
                for name, funcs in tables.items()
            }
        return tables

    hw_specs.get_activation_tables = patched
    bacc_mod.get_activation_tables = patched
    hw_specs._act_tables_patched = True


def _build_graph(b_loc=B_LOC, hw=HW):
    """Build the per-core Bass graph. Returns finalized nc."""
    _patch_act_tables()
    import concourse.bass as bass_mod
    import concourse.bacc as bacc
    import concourse.tile as tile
    from concourse import mybir

    sg_per_b = hw // PIX_PER_SG
    n_sg = b_loc * sg_per_b
    n_mg = n_sg * 2               # pm-groups

    nc = bacc.Bacc()
    x_d = nc.dram_tensor("x", [b_loc, sg_per_b, 128, SGCOLS],
                         mybir.dt.bfloat16, kind="ExternalInput")
    y_d = nc.dram_tensor("y", [b_loc, sg_per_b, IBLK, SGCOLS],
                         mybir.dt.bfloat16, kind="ExternalInput")
    w_d = nc.dram_tensor("w", [128, WCOLS], mybir.dt.bfloat16,
                         kind="ExternalInput")
    cvec_d = nc.dram_tensor("cvec", [128, 1], mybir.dt.float32,
                            kind="ExternalInput")
    o_tp = nc.dram_tensor("o_tp", [8 * b_loc, NCOLS], mybir.dt.float32,
                          kind="ExternalOutput")
    o_ps = nc.dram_tensor("o_ps", [8 * b_loc, NCOLS], mybir.dt.float32,
                          kind="ExternalOutput")
    o_lp = nc.dram_tensor("o_lp", [64, n_mg], mybir.dt.float32,
                          kind="ExternalOutput")

    fp32 = mybir.dt.float32
    bf16 = mybir.dt.bfloat16
    Act = mybir.ActivationFunctionType
    Alu = mybir.AluOpType

    with tile.TileContext(nc) as tc:
        with (
            tc.tile_pool(name="singles", bufs=1) as singles,
            tc.tile_pool(name="xin", bufs=3) as xin,
            tc.tile_pool(name="ybuf", bufs=3) as ybuf,
            tc.tile_pool(name="ebuf", bufs=3) as ebuf,
            tc.tile_pool(name="work", bufs=2) as work,
            tc.tile_pool(name="small", bufs=4) as small,
            tc.tile_pool(name="rdram", bufs=3, space="DRAM") as rdram,
            tc.tile_pool(name="psA", bufs=2, space="PSUM") as psA,
            tc.tile_pool(name="psB", bufs=2, space="PSUM") as psB,
            tc.tile_pool(name="psC", bufs=2, space="PSUM") as psC,
        ):
            w_sb = singles.tile([128, WCOLS], bf16)
            nc.sync.dma_start(out=w_sb, in_=w_d[:, :])
            cvec_sb = singles.tile([128, 1], fp32)
            nc.sync.dma_start(out=cvec_sb, in_=cvec_d[:, :])

            acc_tp = [singles.tile([8, NCOLS], fp32, name=f"acc_tp{b}")
                      for b in range(b_loc)]
            for t in acc_tp:
                nc.vector.memset(t, 0.0)
            acc_lp = singles.tile([64, n_mg], fp32)
            acc_ps = [singles.tile([8, NCOLS], fp32, name=f"acc_ps{b}")
                      for b in range(b_loc)]

            ps_acc = None
            for sgi in range(n_sg):
                b = sgi // sg_per_b
                sg = sgi % sg_per_b

                # -- load x supergroup (one contiguous 2 MiB DMA) --
                xt8 = xin.tile([128, SGCOLS], bf16)
                nc.sync.dma_start(out=xt8, in_=x_d[b, sg])

                # -- y broadcast: one DRAM-replicated (step-0) DMA fills
                #    all 128 partitions (row c*16+i holds y(i,:)) --
                yb = ybuf.tile([128, SGCOLS], bf16, tag="yb")
                ysrc = y_d[b, sg]
                nc.sync.dma_start(
                    out=yb,
                    in_=bass_mod.AP(
                        tensor=ysrc.tensor, offset=ysrc.offset,
                        ap=[[0, C]] + [list(p) for p in ysrc.ap]))

                # -- e8 = exp(x8) --
                e8 = ebuf.tile([128, SGCOLS], bf16)
                nc.scalar.activation(e8, xt8, Act.Exp)

                # -- s8[8i+q, n] = sumexp, 8 accumulated matmuls --
                s8 = psA.tile([128, NCOLS], fp32)
                for q in range(QS):
                    nc.tensor.matmul(
                        s8, w_sb[:, WS0 + 128 * q:WS0 + 128 * (q + 1)],
                        e8[:, NCOLS * q:NCOLS * (q + 1)],
                        start=(q == 0), stop=(q == QS - 1))

                # -- r8 = 1/s8 via exp(-ln) --
                lns8 = small.tile([128, NCOLS], fp32)
                nc.scalar.activation(lns8, s8, Act.Ln)
                r8 = small.tile([128, NCOLS], bf16, tag="r8")
                nc.scalar.activation(r8, lns8, Act.Exp, scale=-1.0)

                # -- r8 -> DRAM scratch; its flat [16, 4096] view is the
                # [i, (q n)] layout (rows are 8i+q); one DRAM-replicated
                # (step-0) DMA broadcasts it to all 128 partitions --
                r_dram = rdram.tile([128, NCOLS], bf16, name="r_dram")
                nc.gpsimd.dma_start(out=r_dram, in_=r8)
                r_flat = r_dram.rearrange("(i q) n -> i (q n)", q=QS)
                rb = ybuf.tile([128, SGCOLS], bf16, tag="rb")
                nc.sync.dma_start(
                    out=rb,
                    in_=bass_mod.AP(
                        tensor=r_flat.tensor, offset=r_flat.offset,
                        ap=[[0, C]] + [list(p) for p in r_flat.ap]))

                # -- mask, p, me over the supergroup [128, 4096] --
                mask8 = work.tile([128, SGCOLS], bf16, tag="mask8")
                nc.vector.tensor_scalar(
                    out=mask8, in0=yb, scalar1=cvec_sb,
                    scalar2=None, op0=Alu.is_equal)
                p8 = work.tile([128, SGCOLS], bf16, tag="p8")
                nc.vector.tensor_mul(p8, e8, rb)
                me8 = work.tile([128, SGCOLS], bf16, tag="me8")
                nc.vector.tensor_mul(me8, p8, mask8)

                # -- pay + tp partials per pm-group of 4 chunks --
                for mg in range(2):
                    pm4 = psB.tile([72, NCOLS], fp32, tag="pm4")
                    for t in range(QM):
                        cq = mg * QM + t
                        nc.tensor.matmul(
                            pm4, w_sb[:, WM0 + 72 * t:WM0 + 72 * (t + 1)],
                            me8[:, NCOLS * cq:NCOLS * (cq + 1)],
                            start=(t == 0), stop=(t == QM - 1))
                    lp_scratch = small.tile([64, NCOLS], bf16, tag="lps")
                    nc.scalar.activation(
                        lp_scratch, pm4[0:64, :], Act.Ln,
                        accum_out=acc_lp[:, 2 * sgi + mg:2 * sgi + mg + 1])
                    nc.vector.tensor_add(acc_tp[b], pm4[64:72, :], acc_tp[b])

                # -- p_sum partials accumulate in PSUM across each batch --
                if sg == 0:
                    ps_acc = psC.tile([8, NCOLS], fp32, tag="ps_acc")
                for q in range(QS):
                    nc.tensor.matmul(
                        ps_acc, w_sb[:, WP0:WP0 + 8],
                        p8[:, NCOLS * q:NCOLS * (q + 1)],
                        start=(sg == 0 and q == 0),
                        stop=(sg == sg_per_b - 1 and q == QS - 1))

                # -- at batch end, move p_sum accumulator out of PSUM --
                if sg == sg_per_b - 1:
                    nc.vector.tensor_copy(acc_ps[b], ps_acc)

            for b in range(b_loc):
                nc.sync.dma_start(out=o_tp[8 * b:8 * b + 8, :], in_=acc_tp[b])
                nc.sync.dma_start(out=o_ps[8 * b:8 * b + 8, :], in_=acc_ps[b])
            nc.sync.dma_start(out=o_lp[:, :], in_=acc_lp)

    nc.finalize()
    return nc


def _host_constants():
    w = np.zeros((128, WCOLS), dtype=_BF16)
    cvec = np.zeros((128, 1), dtype=np.float32)
    for c in range(C):
        for i in range(IBLK):
            k = c * IBLK + i
            cvec[k, 0] = c
            for q in range(QS):
                w[k, WS0 + 128 * q + 8 * i + q] = 1    # ws_q block-diag
            for t in range(QM):
                w[k, WM0 + 72 * t + 4 * i + t] = 1     # wm_t block-diag
                w[k, WM0 + 72 * t + 64 + c] = 1        # wm_t class-sel
            w[k, WP0 + c] = 1                          # wp class-sel
    return w, cvec


def _prep_x(x, hw):
    sg_per_b = hw // PIX_PER_SG
    nb = x.shape[0]
    xr = x.reshape(nb, C, sg_per_b, QS, IBLK, NCOLS)
    return np.ascontiguousarray(
        xr.transpose(0, 2, 1, 4, 3, 5)).reshape(nb, sg_per_b, 128, SGCOLS)


def _prep_y(y_bf, hw):
    sg_per_b = hw // PIX_PER_SG
    nb = y_bf.shape[0]
    yr = y_bf.reshape(nb, sg_per_b, QS, IBLK, NCOLS)
    return np.ascontiguousarray(
        yr.transpose(0, 1, 3, 2, 4)).reshape(nb, sg_per_b, IBLK, SGCOLS)


def kernel(x, y):
    from concourse.bass_utils import run_bass_kernel_spmd

    x = np.asarray(x, dtype=np.float32).reshape(B, C, HW).astype(_BF16)
    y_int = np.asarray(y).reshape(B, HW)
    y_bf = y_int.astype(np.float32).astype(_BF16)

    if "nc" not in _cache:
        _cache["nc"] = _build_graph()
    nc = _cache["nc"]

    w, cvec = _host_constants()
    x_dev = _prep_x(x, HW)
    y_dev = _prep_y(y_bf, HW)
    in_maps = [
        {
            "x": x_dev[j * B_LOC:(j + 1) * B_LOC],
            "y": y_dev[j * B_LOC:(j + 1) * B_LOC],
            "w": w,
            "cvec": cvec,
        }
        for j in range(N_CORES)
    ]
    res = run_bass_kernel_spmd(nc, in_maps, core_ids=list(range(N_CORES)))

    # host-side reduction
    counts = np.stack(
        [np.bincount(y_int[b].astype(np.int64), minlength=C) for b in range(B)]
    ).astype(np.float64)                       # (B, C)

    tp = np.zeros((B, C), dtype=np.float64)
    ps = np.zeros((B, C), dtype=np.float64)
    lp_total = 0.0
    for j in range(N_CORES):
        r = res.results[j]
        otp = np.asarray(r["o_tp"], dtype=np.float64)   # (16, 512)
        ops_ = np.asarray(r["o_ps"], dtype=np.float64)
        olp = np.asarray(r["o_lp"], dtype=np.float64)
        for bl in range(B_LOC):
            bg = j * B_LOC + bl
            tp[bg] = otp[8 * bl:8 * bl + 8].sum(axis=1)
            ps[bg] = ops_[8 * bl:8 * bl + 8].sum(axis=1)
        lp_total += olp.sum()

    dc = (2.0 * tp + SMOOTH) / (ps + counts + SMOOTH + EPS)
    dc_loss = 1.0 - dc[:, 1:].mean()
    ce_loss = -lp_total / (B * HW)
    return np.float32(dc_loss + ce_loss)


# revision 22
# speedup vs baseline: 4.0976x; 1.0223x over previous
"""Dice + CrossEntropy loss kernel for Trainium2 (8 NeuronCores, Bass/Tile).

Problem: x (16, 8, 512, 512) f32 logits, y (16, 512, 512) int labels.
    out = dice_loss + ce_loss   (scalar f32)

Sharding: pure data parallel over the batch dim - core j handles batches
[2j, 2j+1]. All cross-core reductions are tiny and done on the host.

Device pipeline (per core). Work unit is a "supergroup" of 8 chunks; a
chunk is 8192 pixels of one batch image on SBUF layout [128 = (c*16+i),
512 = n], class c in 0..7, pixel-block i in 0..15, pixel = (i, n):

  host: x pre-transposed so a supergroup is one contiguous [128, 4096]
        2 MiB DMA; y to bf16, pre-transposed to [16, 4096].
  ACT : e8 = exp(x8)                                   [128,4096] bf16
  PE  : m1(q), q=0..7, zero-padded block-diag lhsT, accumulate all 8
        chunks' sumexp into one PSUM tile s8[128,512] (row 8i+q).
  ACT : lns8 = ln(s8); r8 = exp(-lns8) = 1/s           [128,512]
  DMA : r8 -> DRAM scratch -> permuted into yrg r-half rows 0:16
        (DRAM APs are unconstrained); then 3 doubling SBUF copies
        broadcast [y | r] rows 0:16 -> all 128 partitions.
  DVE : mask8 = (y == c); p8 = e8 * r; me8 = p8 * mask8  [128,4096]
  PE  : m3(t) per 4-chunk pm-group -> pm4[72,512]: rows 4i+t = p@y
        per pixel, rows 64:72 = tp partials; m4(q) -> ps_acc[8,512]
        p_sum partials accumulated across each batch.
  ACT : ln(pm4 rows 0:64) with accum_out -> CE partial column
  DVE : acc_tp += tp partials; batch end: copy ps_acc out of PSUM
Host: tiny final reductions + dice/CE formula; label counts via bincount.
"""

import os
import sys

if os.path.isdir("/opt/trn_rl_repo") and "/opt/trn_rl_repo" not in sys.path:
    sys.path.insert(0, "/opt/trn_rl_repo")

import numpy as np
import ml_dtypes

B, C, H, W = 16, 8, 512, 512
HW = H * W
N_CORES = 8
B_LOC = B // N_CORES          # batches per core
SMOOTH = 1e-05
EPS = 1e-08

NCOLS = 512                   # free-dim columns per chunk
IBLK = 16                     # pixel blocks per chunk
PIX_PER_CHUNK = IBLK * NCOLS  # 8192
QS = 8                        # chunks per supergroup
SGCOLS = QS * NCOLS           # 4096
PIX_PER_SG = QS * PIX_PER_CHUNK  # 65536
QM = 4                        # chunks per pm-group (pay/tp matmul group)

_BF16 = ml_dtypes.bfloat16

_cache = {}

# wpack column layout
WS0 = 0                # 8 blocks of [128,128] at cols 128*q
WM0 = 8 * 128          # 4 blocks of [128,72] at cols WM0 + 72*t
WP0 = WM0 + 4 * 72     # [128,8]
WCOLS = WP0 + 8        # 1320


def _patch_act_tables():
    """Force every activation onto the one table set that contains both
    Exp and Ln, so the kernel needs a single ACT_TABLE_LOAD instead of
    thrashing tables between Exp and Ln every chunk. Index order of the
    sets is preserved (ids must match act_info.json); unwanted sets are
    just emptied so the chooser can't pick them."""
    from concourse import hw_specs
    import concourse.bacc as bacc_mod

    if getattr(hw_specs, "_act_tables_patched", False):
        return
    orig = hw_specs.get_activation_tables

    def patched(arch):
        tables = orig(arch)
        keep = "natural_log_exp_and_others"
        if keep in tables:
            tables = {
                name: (funcs if name == keep else set())
                for name, funcs in tables.items()
            }
        return tables

    hw_specs.get_activation_tables = patched
    bacc_mod.get_activation_tables = patched
    hw_specs._act_tables_patched = True


def _build_graph(b_loc=B_LOC, hw=HW):
    """Build the per-core Bass graph. Returns finalized nc."""
    _patch_act_tables()
    import concourse.bass as bass_mod
    import concourse.bacc as bacc
    import concourse.tile as tile
    from concourse import mybir

    sg_per_b = hw // PIX_PER_SG
    n_sg = b_loc * sg_per_b
    n_mg = n_sg * 2               # pm-groups

    nc = bacc.Bacc()
    x_d = nc.dram_tensor("x", [b_loc, sg_per_b, 128, SGCOLS],
                         mybir.dt.bfloat16, kind="ExternalInput")
    y_d = nc.dram_tensor("y", [b_loc, sg_per_b, IBLK, SGCOLS],
                         mybir.dt.bfloat16, kind="ExternalInput")
    w_d = nc.dram_tensor("w", [128, WCOLS], mybir.dt.bfloat16,
                         kind="ExternalInput")
    cvec_d = nc.dram_tensor("cvec", [128, 1], mybir.dt.float32,
                            kind="ExternalInput")
    o_tp = nc.dram_tensor("o_tp", [8 * b_loc, NCOLS], mybir.dt.float32,
                          kind="ExternalOutput")
    o_ps = nc.dram_tensor("o_ps", [128, n_sg], mybir.dt.float32,
                          kind="ExternalOutput")
    o_lp = nc.dram_tensor("o_lp", [64, n_mg], mybir.dt.float32,
                          kind="ExternalOutput")

    fp32 = mybir.dt.float32
    bf16 = mybir.dt.bfloat16
    Act = mybir.ActivationFunctionType
    Alu = mybir.AluOpType

    with tile.TileContext(nc) as tc:
        with (
            tc.tile_pool(name="singles", bufs=1) as singles,
            tc.tile_pool(name="xin", bufs=3) as xin,
            tc.tile_pool(name="ybuf", bufs=3) as ybuf,
            tc.tile_pool(name="ebuf", bufs=3) as ebuf,
            tc.tile_pool(name="work", bufs=2) as work,
            tc.tile_pool(name="small", bufs=4) as small,
            tc.tile_pool(name="rdram", bufs=3, space="DRAM") as rdram,
            tc.tile_pool(name="psA", bufs=2, space="PSUM") as psA,
            tc.tile_pool(name="psB", bufs=2, space="PSUM") as psB,
        ):
            w_sb = singles.tile([128, WCOLS], bf16)
            nc.sync.dma_start(out=w_sb, in_=w_d[:, :])
            cvec_sb = singles.tile([128, 1], fp32)
            nc.sync.dma_start(out=cvec_sb, in_=cvec_d[:, :])

            acc_tp = [singles.tile([8, NCOLS], fp32, name=f"acc_tp{b}")
                      for b in range(b_loc)]
            for t in acc_tp:
                nc.vector.memset(t, 0.0)
            acc_lp = singles.tile([64, n_mg], fp32)
            acc_ps = singles.tile([128, n_sg], fp32)

            for sgi in range(n_sg):
                b = sgi // sg_per_b
                sg = sgi % sg_per_b

                # -- load x supergroup (one contiguous 2 MiB DMA) --
                xt8 = xin.tile([128, SGCOLS], bf16)
                nc.sync.dma_start(out=xt8, in_=x_d[b, sg])

                # -- y broadcast: one DRAM-replicated (step-0) DMA fills
                #    all 128 partitions (row c*16+i holds y(i,:)) --
                yb = ybuf.tile([128, SGCOLS], bf16, tag="yb")
                ysrc = y_d[b, sg]
                nc.sync.dma_start(
                    out=yb,
                    in_=bass_mod.AP(
                        tensor=ysrc.tensor, offset=ysrc.offset,
                        ap=[[0, C]] + [list(p) for p in ysrc.ap]))

                # -- e8 = exp(x8) --
                e8 = ebuf.tile([128, SGCOLS], bf16)
                nc.scalar.activation(e8, xt8, Act.Exp)

                # -- s8[8i+q, n] = sumexp, 8 accumulated matmuls --
                s8 = psA.tile([128, NCOLS], fp32)
                for q in range(QS):
                    nc.tensor.matmul(
                        s8, w_sb[:, WS0 + 128 * q:WS0 + 128 * (q + 1)],
                        e8[:, NCOLS * q:NCOLS * (q + 1)],
                        start=(q == 0), stop=(q == QS - 1))

                # -- r8 = 1/s8 via exp(-ln) --
                lns8 = small.tile([128, NCOLS], fp32)
                nc.scalar.activation(lns8, s8, Act.Ln)
                r8 = small.tile([128, NCOLS], bf16, tag="r8")
                nc.scalar.activation(r8, lns8, Act.Exp, scale=-1.0)

                # -- r8 -> DRAM scratch; its flat [16, 4096] view is the
                # [i, (q n)] layout (rows are 8i+q); one DRAM-replicated
                # (step-0) DMA broadcasts it to all 128 partitions --
                r_dram = rdram.tile([128, NCOLS], bf16, name="r_dram")
                nc.gpsimd.dma_start(out=r_dram, in_=r8)
                r_flat = r_dram.rearrange("(i q) n -> i (q n)", q=QS)
                rb = ybuf.tile([128, SGCOLS], bf16, tag="rb")
                nc.sync.dma_start(
                    out=rb,
                    in_=bass_mod.AP(
                        tensor=r_flat.tensor, offset=r_flat.offset,
                        ap=[[0, C]] + [list(p) for p in r_flat.ap]))

                # -- mask, p (fused with p_sum reduce), me --
                mask8 = work.tile([128, SGCOLS], bf16, tag="mask8")
                nc.vector.tensor_scalar(
                    out=mask8, in0=yb, scalar1=cvec_sb,
                    scalar2=None, op0=Alu.is_equal)
                p8 = work.tile([128, SGCOLS], bf16, tag="p8")
                nc.vector.tensor_mul(p8, e8, rb)
                nc.vector.reduce_sum(acc_ps[:, sgi:sgi + 1], p8,
                                     axis=mybir.AxisListType.X)
                me8 = work.tile([128, SGCOLS], bf16, tag="me8")
                nc.vector.tensor_mul(me8, p8, mask8)

                # -- pay + tp partials per pm-group of 4 chunks --
                for mg in range(2):
                    pm4 = psB.tile([72, NCOLS], fp32, tag="pm4")
                    for t in range(QM):
                        cq = mg * QM + t
                        nc.tensor.matmul(
                            pm4, w_sb[:, WM0 + 72 * t:WM0 + 72 * (t + 1)],
                            me8[:, NCOLS * cq:NCOLS * (cq + 1)],
                            start=(t == 0), stop=(t == QM - 1))
                    lp_scratch = small.tile([64, NCOLS], bf16, tag="lps")
                    nc.scalar.activation(
                        lp_scratch, pm4[0:64, :], Act.Ln,
                        accum_out=acc_lp[:, 2 * sgi + mg:2 * sgi + mg + 1])
                    nc.vector.tensor_add(acc_tp[b], pm4[64:72, :], acc_tp[b])

            for b in range(b_loc):
                nc.sync.dma_start(out=o_tp[8 * b:8 * b + 8, :], in_=acc_tp[b])
            nc.sync.dma_start(out=o_ps[:, :], in_=acc_ps)
            nc.sync.dma_start(out=o_lp[:, :], in_=acc_lp)

    nc.finalize()
    return nc


def _host_constants():
    w = np.zeros((128, WCOLS), dtype=_BF16)
    cvec = np.zeros((128, 1), dtype=np.float32)
    for c in range(C):
        for i in range(IBLK):
            k = c * IBLK + i
            cvec[k, 0] = c
            for q in range(QS):
                w[k, WS0 + 128 * q + 8 * i + q] = 1    # ws_q block-diag
            for t in range(QM):
                w[k, WM0 + 72 * t + 4 * i + t] = 1     # wm_t block-diag
                w[k, WM0 + 72 * t + 64 + c] = 1        # wm_t class-sel
            w[k, WP0 + c] = 1                          # wp class-sel
    return w, cvec


def _prep_x(x, hw):
    sg_per_b = hw // PIX_PER_SG
    nb = x.shape[0]
    xr = x.reshape(nb, C, sg_per_b, QS, IBLK, NCOLS)
    return np.ascontiguousarray(
        xr.transpose(0, 2, 1, 4, 3, 5)).reshape(nb, sg_per_b, 128, SGCOLS)


def _prep_y(y_bf, hw):
    sg_per_b = hw // PIX_PER_SG
    nb = y_bf.shape[0]
    yr = y_bf.reshape(nb, sg_per_b, QS, IBLK, NCOLS)
    return np.ascontiguousarray(
        yr.transpose(0, 1, 3, 2, 4)).reshape(nb, sg_per_b, IBLK, SGCOLS)


def kernel(x, y):
    from concourse.bass_utils import run_bass_kernel_spmd

    x = np.asarray(x, dtype=np.float32).reshape(B, C, HW).astype(_BF16)
    y_int = np.asarray(y).reshape(B, HW)
    y_bf = y_int.astype(np.float32).astype(_BF16)

    if "nc" not in _cache:
        _cache["nc"] = _build_graph()
    nc = _cache["nc"]

    w, cvec = _host_constants()
    x_dev = _prep_x(x, HW)
    y_dev = _prep_y(y_bf, HW)
    in_maps = [
        {
            "x": x_dev[j * B_LOC:(j + 1) * B_LOC],
            "y": y_dev[j * B_LOC:(j + 1) * B_LOC],
            "w": w,
            "cvec": cvec,
        }
        for j in range(N_CORES)
    ]
    res = run_bass_kernel_spmd(nc, in_maps, core_ids=list(range(N_CORES)))

    # host-side reduction
    counts = np.stack(
        [np.bincount(y_int[b].astype(np.int64), minlength=C) for b in range(B)]
    ).astype(np.float64)                       # (B, C)

    tp = np.zeros((B, C), dtype=np.float64)
    ps = np.zeros((B, C), dtype=np.float64)
    lp_total = 0.0
    for j in range(N_CORES):
        r = res.results[j]
        otp = np.asarray(r["o_tp"], dtype=np.float64)   # (16, 512)
        ops_ = np.asarray(r["o_ps"], dtype=np.float64)  # (128, n_sg)
        olp = np.asarray(r["o_lp"], dtype=np.float64)
        n_sg_b = ops_.shape[1] // B_LOC
        for bl in range(B_LOC):
            bg = j * B_LOC + bl
            tp[bg] = otp[8 * bl:8 * bl + 8].sum(axis=1)
            ps[bg] = ops_[:, bl * n_sg_b:(bl + 1) * n_sg_b].sum(
                axis=1).reshape(C, IBLK).sum(axis=1)
        lp_total += olp.sum()

    dc = (2.0 * tp + SMOOTH) / (ps + counts + SMOOTH + EPS)
    dc_loss = 1.0 - dc[:, 1:].mean()
    ce_loss = -lp_total / (B * HW)
    return np.float32(dc_loss + ce_loss)
